# revision 1
# baseline (speedup 1.0000x reference)
"""Distributed Trainium2 kernel for the linear-attention transformer block.

Math (per batch element b):
  Q = elu(x @ Wq + bq), K = elu(x @ Wk + bk), V = x @ Wv + bv   (per-head d=64)
  KV_h = K_h^T V_h  [64,64];  Ksum_h = sum_n K_h[n]  [64]
  attn_h = (Q_h @ KV_h) / (Q_h . Ksum_h)
  out = LayerNorm(x + attn @ Wo + bo) * gamma + beta

Sharding: 16384 tokens over 8 cores (2048 each; core c owns batch c//2,
half c%2). Each core computes Q/K/V only for its tokens, partial KV/Ksum,
then a 266KB AllReduce over core pairs {2b, 2b+1} completes the KV stats;
attention + output projection + LayerNorm finish locally.

Precision: big matmuls in bf16. Q/K projections use a 3-term bf16 split
(x_hi@W_hi + x_hi@W_lo + x_lo@W_hi) because the 1/(Q.Ksum) denominators
pass near zero and amplify bf16 noise. Ksum and the denominator run in
f32. Validated ~1.3e-3 global rel err vs the f32 reference.
"""

import sys

sys.path.insert(0, "/opt/trn_rl_repo")

import numpy as np
import ml_dtypes

import concourse.bass as bass
import concourse.mybir as mybir
import concourse.tile as tile
from concourse import bacc
from concourse.bass_utils import run_bass_kernel_spmd

AF = mybir.ActivationFunctionType
OP = mybir.AluOpType
F32 = mybir.dt.float32
BF16 = mybir.dt.bfloat16

B, N, D = 4, 4096, 1024
H, HD = 16, 64
TOK = 2048            # tokens per core
NCORES = 8
LN_EPS = 1e-3
P = 128
KC = D // P           # 8 contraction chunks
TC = TOK // P         # 16 token chunks of 128
TQ = TOK // 512       # 4 token chunks of 512

LAST_RESULT = None    # BassKernelResults of the most recent run (for test.py)
DEBUG_TAPS = False    # set True (e.g. from debug.py) to add intermediate outputs


def _build(apply_bias, apply_gamma, apply_beta):
    nc = bacc.Bacc("TRN2", target_bir_lowering=False, debug=False, num_devices=NCORES)

    def din(name, shape, dtype=BF16):
        return nc.dram_tensor(name, shape, dtype, kind="ExternalInput")

    xthi = din("xthi", [D, TOK])
    xtlo = din("xtlo", [D, TOK])
    wqh = din("wqh", [D, D])
    wql = din("wql", [D, D])
    wkh = din("wkh", [D, D])
    wkl = din("wkl", [D, D])
    wvh = din("wvh", [D, D])
    woh = din("woh", [D, D])
    xres = din("xres", [TOK, D], F32)
    e_sel = din("e_sel", [2, P])
    if apply_bias:
        bq_d = din("bq", [D], F32)
        bk_d = din("bk", [D], F32)
        bv_d = din("bv", [D], F32)
        bo_d = din("bo", [D], F32)
    if apply_gamma:
        gamma_d = din("gamma", [D], F32)
    if apply_beta:
        beta_d = din("beta", [D], F32)
    out_d = nc.dram_tensor("out", [TOK, D], F32, kind="ExternalOutput")
    if DEBUG_TAPS:
        dbg_k0 = nc.dram_tensor("dbg_k0", [P, 512], F32, kind="ExternalOutput")
        dbg_kv = nc.dram_tensor("dbg_kv", [P, 512], F32, kind="ExternalOutput")
        dbg_ksum = nc.dram_tensor("dbg_ksum", [P, KC], F32, kind="ExternalOutput")
        dbg_ar = nc.dram_tensor("dbg_ar", [P, 520], F32, kind="ExternalOutput")
        dbg_qt0 = nc.dram_tensor("dbg_qt0", [P, TOK], F32, kind="ExternalOutput")
        dbg_at = nc.dram_tensor("dbg_at", [P, KC, TOK], F32, kind="ExternalOutput")

    r8 = lambda t: t.ap().rearrange("(ko p) n -> p ko n", p=P)

    def bcast_row(dram_vec, sb_tile):
        # DMA-broadcast a [D] vector to [P, D] (stride-0 partition dim).
        src = bass.AP(
            tensor=dram_vec.ap().tensor,
            offset=dram_vec.ap().offset,
            ap=[[0, P]] + list(dram_vec.ap().ap),
        )
        nc.sync.dma_start(out=sb_tile, in_=src)

    with tile.TileContext(nc) as tc:
        with (
            tc.tile_pool(name="xpool", bufs=1) as xpool,
            tc.tile_pool(name="smalls", bufs=1) as smalls,
            tc.tile_pool(name="dram", bufs=1, space="DRAM") as dram,
        ):
            # ---- resident x^T (hi/lo bf16 split) ----
            xthi_sb = xpool.tile([P, KC, TOK], BF16)
            xtlo_sb = xpool.tile([P, KC, TOK], BF16)

            e_sb = smalls.tile([2, P], BF16)
            nc.sync.dma_start(e_sb[:], e_sel.ap())
            ones_sb = smalls.tile([P, 1], F32)
            nc.vector.memset(ones_sb[:], 1.0)
            eps_sb = smalls.tile([P, 1], F32)
            nc.vector.memset(eps_sb[:], LN_EPS)
            if apply_bias:
                bq_sb = smalls.tile([P, KC], F32)   # per-partition layout for Q^T
                nc.sync.dma_start(bq_sb[:], bq_d.ap().rearrange("(ko p) -> p ko", p=P))
                bk_b = smalls.tile([P, D], F32)
                bv_b = smalls.tile([P, D], F32)
                bo_b = smalls.tile([P, D], F32)
                bcast_row(bk_d, bk_b[:])
                bcast_row(bv_d, bv_b[:])
                bcast_row(bo_d, bo_b[:])
            if apply_gamma:
                gamma_b = smalls.tile([P, D], F32)
                bcast_row(gamma_d, gamma_b[:])
            if apply_beta:
                beta_b = smalls.tile([P, D], F32)
                bcast_row(beta_d, beta_b[:])

            if DEBUG_TAPS:
                kv_loc = smalls.tile([P, 512], F32)
                ksum_loc = smalls.tile([P, KC], F32)

            # PE warmup: matmuls on zeroed tiles release the HAM clock
            # gate (1.2->2.4 GHz) while the first input DMAs are in flight.
            with (
                tc.tile_pool(name="warmsb", bufs=1) as warmsb,
                tc.tile_pool(name="warmps", bufs=2, space="PSUM") as warmps,
            ):
                warm_a = warmsb.tile([P, P], BF16)
                warm_b = warmsb.tile([P, 512], BF16)
                nc.gpsimd.memset(warm_a[:], 0.0)
                nc.gpsimd.memset(warm_b[:], 0.0)
                for w in range(64):
                    wp = warmps.tile([P, 512], F32, tag="warm", name=f"warm_{w}")
                    nc.tensor.matmul(wp[:], warm_a[:], warm_b[:], start=True, stop=True)

            # Prefetch the first two Q-weight slices; their DMAs have no deps
            # and fill otherwise-idle DMA time during phase 1.
            wqp_cm = tc.tile_pool(name="wqp", bufs=3)
            wqp = wqp_cm.__enter__()
            wq_tiles = {}

            def load_wq(hp):
                msl = slice(hp * P, (hp + 1) * P)
                wq_t = wqp.tile([P, KC, 2, P], BF16, tag="wq", name=f"wq_{hp}")
                nc.sync.dma_start(wq_t[:, :, 0, :], r8(wqh)[:, :, msl])
                nc.sync.dma_start(wq_t[:, :, 1, :], r8(wql)[:, :, msl])
                wq_tiles[hp] = wq_t

            # ================= Phase 1: K, V, partial KV + Ksum =================
            with (
                tc.tile_pool(name="wkv", bufs=1) as wkv,
                tc.tile_pool(name="kvps_pool", bufs=1, space="PSUM") as kvps_pool,
                tc.tile_pool(name="ph1ps", bufs=7, space="PSUM") as ph1ps,
                tc.tile_pool(name="ph1sb", bufs=4) as ph1sb,
            ):
                wkh_sb = wkv.tile([P, KC, D], BF16)
                wkl_sb = wkv.tile([P, KC, D], BF16)
                wvh_sb = wkv.tile([P, KC, D], BF16)
                for k in range(KC):
                    nc.sync.dma_start(xthi_sb[:, k, :], r8(xthi)[:, k, :])
                    nc.sync.dma_start(wkh_sb[:, k, :], r8(wkh)[:, k, :])
                    nc.sync.dma_start(wkl_sb[:, k, :], r8(wkl)[:, k, :])
                    nc.sync.dma_start(xtlo_sb[:, k, :], r8(xtlo)[:, k, :])
                    nc.sync.dma_start(wvh_sb[:, k, :], r8(wvh)[:, k, :])
                load_wq(0)
                load_wq(1)

                # SBUF accumulator (DVE-add per token chunk; interleaved
                # multi-chunk PSUM accumulation groups proved unreliable).
                # Layout per dh half: [dh*260, dh*260+256) = KV, [+256, +260) = Ksum.
                acc = smalls.tile([P, 520], F32)
                nc.vector.memset(acc[:], 0.0)

                for t in range(TC):
                    ts = slice(t * P, (t + 1) * P)
                    kb_chunks = []
                    kvs_tiles = {}
                    for dh in range(2):
                        dsl = slice(dh * 512, (dh + 1) * 512)
                        kps = ph1ps.tile([P, 512], F32, tag="proj", name=f"kps_{t}_{dh}")
                        for k in range(KC):
                            nc.tensor.matmul(kps[:], xthi_sb[:, k, ts], wkh_sb[:, k, dsl],
                                             start=(k == 0), stop=False)
                            nc.tensor.matmul(kps[:], xthi_sb[:, k, ts], wkl_sb[:, k, dsl],
                                             start=False, stop=False)
                            nc.tensor.matmul(kps[:], xtlo_sb[:, k, ts], wkh_sb[:, k, dsl],
                                             start=False, stop=(k == KC - 1))
                        if apply_bias:
                            kraw = ph1sb.tile([P, 512], F32, tag="kraw", name=f"kraw_{t}_{dh}")
                            nc.vector.tensor_tensor(kraw[:], kps[:], bk_b[:, dsl], OP.add)
                            ksrc = kraw
                        else:
                            ksrc = kps
                        kmin = ph1sb.tile([P, 512], F32, tag="kmin", name=f"kmin_{t}_{dh}")
                        nc.vector.tensor_scalar(kmin[:], ksrc[:], 0.0, None, OP.min)
                        kexp = ph1sb.tile([P, 512], F32, tag="kexp", name=f"kexp_{t}_{dh}")
                        nc.scalar.activation(kexp[:], kmin[:], AF.Exp)
                        kmax = ph1sb.tile([P, 512], F32, tag="kmax", name=f"kmax_{t}_{dh}")
                        nc.vector.tensor_scalar(kmax[:], ksrc[:], 0.0, -1.0, OP.max, OP.add)
                        kf = ph1sb.tile([P, 512], F32, tag="kf", name=f"kf_{t}_{dh}")
                        nc.vector.tensor_tensor(kf[:], kmax[:], kexp[:], OP.add)
                        kb = ph1sb.tile([P, 512], BF16, tag="kb", name=f"kb_{t}_{dh}")
                        nc.vector.tensor_copy(kb[:], kf[:])
                        if DEBUG_TAPS and t == 0 and dh == 0:
                            nc.sync.dma_start(dbg_k0.ap(), kf[:])
                        kb_chunks.append(kb)
                        # Ksum column blocks (f32 matmul against ones) go into
                        # cols [256, 260) of the shared kvs_t psum tile.
                        kvs_t = kvps_pool.tile([P, 260], F32, tag="kvs_t",
                                               name=f"kvs_t_{t}_{dh}", bufs=1)
                        kvs_tiles[dh] = kvs_t
                        for j in range(4):
                            nc.tensor.matmul(
                                kvs_t[:, 256 + j:257 + j], kf[:, j * P:(j + 1) * P],
                                ones_sb[:], start=True, stop=True, skip_group_check=True)
                    for dh in range(2):
                        dsl = slice(dh * 512, (dh + 1) * 512)
                        vps = ph1ps.tile([P, 512], F32, tag="proj", name=f"vps_{t}_{dh}")
                        for k in range(KC):
                            nc.tensor.matmul(vps[:], xthi_sb[:, k, ts], wvh_sb[:, k, dsl],
                                             start=(k == 0), stop=(k == KC - 1))
                        vb = ph1sb.tile([P, 512], BF16, tag="vb", name=f"vb_{t}_{dh}")
                        if apply_bias:
                            nc.vector.tensor_tensor(vb[:], vps[:], bv_b[:, dsl], OP.add)
                        else:
                            nc.any.tensor_copy(vb[:], vps[:])
                        kb = kb_chunks[dh]
                        kvs_t = kvs_tiles[dh]
                        for hh in range(8):
                            h = dh * 8 + hh
                            pr = (h % 2) * 64
                            fc = (h // 2) * 64 - dh * 256
                            nc.tensor.matmul(
                                kvs_t[pr:pr + 64, fc:fc + 64],
                                kb[:, hh * 64:(hh + 1) * 64],
                                vb[:, hh * 64:(hh + 1) * 64],
                                start=True, stop=True,
                                tile_position=(0, pr), skip_group_check=True)
                        nc.vector.tensor_tensor(
                            acc[:, dh * 260:(dh + 1) * 260],
                            acc[:, dh * 260:(dh + 1) * 260], kvs_t[:], OP.add)

                if DEBUG_TAPS:
                    nc.vector.tensor_copy(kv_loc[:, :256], acc[:, :256])
                    nc.vector.tensor_copy(kv_loc[:, 256:], acc[:, 260:516])
                    nc.vector.tensor_copy(ksum_loc[:, :4], acc[:, 256:260])
                    nc.vector.tensor_copy(ksum_loc[:, 4:], acc[:, 516:520])
                    nc.sync.dma_start(dbg_kv.ap(), kv_loc[:])
                    nc.sync.dma_start(dbg_ksum.ap(), ksum_loc[:])

            # ========== Phases 2-4: AllReduce; Q^T; attention (pipelined) ==========
            with tc.tile_pool(name="late", bufs=1) as late:
                at_sb = late.tile([P, KC, TOK], BF16)
                woh_sb = late.tile([P, KC, D], BF16)

                with (
                    tc.tile_pool(name="qtp", bufs=4) as qtp,
                    tc.tile_pool(name="ph3ps", bufs=3, space="PSUM") as ph3ps,
                    tc.tile_pool(name="ph3sb", bufs=3) as ph3sb,
                    tc.tile_pool(name="ph4ps_d", bufs=1, space="PSUM") as ph4ps_d,
                    tc.tile_pool(name="ph4ps_z", bufs=2, space="PSUM") as ph4ps_z,
                    tc.tile_pool(name="ph4ps_a", bufs=2, space="PSUM") as ph4ps_a,
                    tc.tile_pool(name="ph4sb", bufs=4) as ph4sb,
                ):
                    qt_tiles = {}

                    # -- AllReduce of the packed KV/Ksum accumulator --
                    cc_in = dram.tile([P, 520], F32)
                    cc_out = dram.tile([P, 520], F32)
                    nc.sync.dma_start(cc_in[:], acc[:])
                    nc.gpsimd.collective_compute(
                        "AllReduce", OP.add,
                        replica_groups=[[0, 1], [2, 3], [4, 5], [6, 7]],
                        ins=[cc_in[:].opt()], outs=[cc_out[:].opt()])
                    ar_sb = smalls.tile([P, 520], F32)
                    nc.sync.dma_start(ar_sb[:], cc_out[:])
                    if DEBUG_TAPS:
                        nc.sync.dma_start(dbg_ar.ap(), ar_sb[:])
                    kv_bf = smalls.tile([P, 512], BF16)
                    nc.gpsimd.tensor_copy(kv_bf[:, :256], ar_sb[:, :256])
                    nc.gpsimd.tensor_copy(kv_bf[:, 256:], ar_sb[:, 260:516])
                    kd_sb = smalls.tile([P, H], F32)  # col h: Ksum_h at rows (h%2)*64
                    nc.gpsimd.memset(kd_sb[:], 0.0)
                    for h in range(H):
                        pr = (h % 2) * 64
                        c = h // 2
                        sc = 256 + c if c < 4 else 516 + (c - 4)
                        nc.gpsimd.tensor_copy(
                            kd_sb[pr:pr + 64, h:h + 1], ar_sb[pr:pr + 64, sc:sc + 1])

                    for k in range(KC):
                        nc.sync.dma_start(woh_sb[:, k, :], r8(woh)[:, k, :])

                    def q_proj(hp):
                        wq_t = wq_tiles.pop(hp)
                        qt = qtp.tile([P, TOK], F32, tag="qt", name=f"qt_{hp}")
                        qt_tiles[hp] = qt
                        for tq in range(TQ):
                            tsl = slice(tq * 512, (tq + 1) * 512)
                            qps = ph3ps.tile([P, 512], F32, tag="qps", name=f"qps_{hp}_{tq}")
                            for k in range(KC):
                                nc.tensor.matmul(qps[:], wq_t[:, k, 0, :], xthi_sb[:, k, tsl],
                                                 start=(k == 0), stop=False)
                                nc.tensor.matmul(qps[:], wq_t[:, k, 1, :], xthi_sb[:, k, tsl],
                                                 start=False, stop=False)
                                nc.tensor.matmul(qps[:], wq_t[:, k, 0, :], xtlo_sb[:, k, tsl],
                                                 start=False, stop=(k == KC - 1))
                            if apply_bias:
                                qraw = ph3sb.tile([P, 512], F32, tag="qraw", name=f"qraw_{hp}_{tq}")
                                nc.vector.tensor_scalar(qraw[:], qps[:], bq_sb[:, hp:hp + 1],
                                                        None, OP.add)
                                qsrc = qraw
                            else:
                                qsrc = qps
                            qmin = ph3sb.tile([P, 512], F32, tag="qmin", name=f"qmin_{hp}_{tq}")
                            nc.vector.tensor_scalar(qmin[:], qsrc[:], 0.0, None, OP.min)
                            qexp = ph3sb.tile([P, 512], F32, tag="qexp", name=f"qexp_{hp}_{tq}")
                            nc.scalar.activation(qexp[:], qmin[:], AF.Exp)
                            qmax = ph3sb.tile([P, 512], F32, tag="qmax", name=f"qmax_{hp}_{tq}")
                            nc.vector.tensor_scalar(qmax[:], qsrc[:], 0.0, -1.0, OP.max, OP.add)
                            nc.vector.tensor_tensor(qt[:, tsl], qmax[:], qexp[:], OP.add)

                    def attention(hp):
                        qt = qt_tiles.pop(hp)
                        if DEBUG_TAPS and hp == 0:
                            nc.sync.dma_start(dbg_qt0.ap(), qt[:])
                        for tq in range(TQ):
                            tsl = slice(tq * 512, (tq + 1) * 512)
                            dps = ph4ps_d.tile([2, 512], F32, tag="dps", name=f"dps_{hp}_{tq}")
                            nc.tensor.matmul(dps[:], kd_sb[:, 2 * hp:2 * hp + 2],
                                             qt[:, tsl], start=True, stop=True)
                            zrf = ph4sb.tile([2, 512], F32, tag="zrf", name=f"zrf_{hp}_{tq}")
                            nc.vector.reciprocal(zrf[:], dps[:])
                            zr = ph4sb.tile([2, 512], BF16, tag="zr", name=f"zr_{hp}_{tq}")
                            nc.vector.tensor_copy(zr[:], zrf[:])
                            zps = ph4ps_z.tile([P, 512], F32, tag="zps", name=f"zps_{hp}_{tq}")
                            nc.tensor.matmul(zps[:], e_sb[:], zr[:], start=True, stop=True)
                            zf = ph4sb.tile([P, 512], F32, tag="zf", name=f"zf_{hp}_{tq}")
                            nc.vector.tensor_copy(zf[:], zps[:])
                            qbf = ph4sb.tile([P, 512], BF16, tag="qbf", name=f"qbf_{hp}_{tq}")
                            nc.vector.tensor_copy(qbf[:], qt[:, tsl])
                            aps = ph4ps_a.tile([P, 512], F32, tag="aps", name=f"aps_{hp}_{tq}")
                            for par in (0, 64):
                                nc.tensor.matmul(
                                    aps[par:par + 64, :],
                                    kv_bf[par:par + 64, hp * 64:(hp + 1) * 64],
                                    qbf[par:par + 64, :],
                                    start=True, stop=True, tile_position=(par, par))
                            nc.vector.tensor_tensor(at_sb[:, hp, tsl], aps[:], zf[:], OP.mult)

                    # depth-2 software pipeline: attention(hp) runs two Q chunks
                    # behind, so the AllReduce hides under ~3 Q projections.
                    q_proj(0)
                    for hp in range(1, KC):
                        if hp + 1 < KC:
                            load_wq(hp + 1)
                        q_proj(hp)
                        if hp >= 2:
                            attention(hp - 2)
                    attention(KC - 2)
                    attention(KC - 1)

                if DEBUG_TAPS:
                    with tc.tile_pool(name="dbgat", bufs=2) as dbgat:
                        for c in range(KC):
                            atf = dbgat.tile([P, TOK], F32, tag="atf", name=f"atf_{c}")
                            nc.vector.tensor_copy(atf[:], at_sb[:, c, :])
                            nc.sync.dma_start(dbg_at.ap()[:, c, :], atf[:])

                # ===== Phase 5: output projection + residual + LayerNorm =====
                with (
                    tc.tile_pool(name="ph5ps", bufs=3, space="PSUM") as ph5ps,
                    tc.tile_pool(name="ph5sb", bufs=3) as ph5sb,
                ):
                    for t in range(TC):
                        ts = slice(t * P, (t + 1) * P)
                        y = ph5sb.tile([P, D], F32, tag="y", name=f"y_{t}")
                        xr = ph5sb.tile([P, D], F32, tag="xr", name=f"xr_{t}")
                        nc.sync.dma_start(xr[:], xres.ap()[ts, :])
                        ops = ph5ps.tile([P, D], F32, tag="ops", name=f"ops_{t}")
                        for dh in range(2):
                            dsl = slice(dh * 512, (dh + 1) * 512)
                            for c in range(KC):
                                nc.tensor.matmul(ops[:, dsl], at_sb[:, c, ts], woh_sb[:, c, dsl],
                                                 start=(c == 0), stop=(c == KC - 1))
                        nc.vector.tensor_tensor(y[:], ops[:], xr[:], OP.add)
                        if apply_bias:
                            nc.vector.tensor_tensor(y[:], y[:], bo_b[:], OP.add)
                        stats = ph5sb.tile([P, 2, 6], F32, tag="stats", name=f"stats_{t}")
                        nc.vector.bn_stats(out=stats[:, 0, :], in_=y[:, :512])
                        nc.vector.bn_stats(out=stats[:, 1, :], in_=y[:, 512:])
                        mv = ph5sb.tile([P, 2], F32, tag="mv", name=f"mv_{t}")
                        nc.vector.bn_aggr(out=mv[:], in_=stats[:])
                        nc.scalar.activation(out=mv[:, 1:2], in_=mv[:, 1:2], func=AF.Sqrt,
                                             bias=eps_sb[:], scale=1.0)
                        nc.vector.reciprocal(mv[:, 1:2], mv[:, 1:2])
                        yo = ph5sb.tile([P, D], F32, tag="yo", name=f"yo_{t}")
                        nc.gpsimd.tensor_scalar(yo[:], y[:], mv[:, 0:1], mv[:, 1:2],
                                                OP.subtract, OP.mult)
                        if apply_gamma:
                            nc.vector.tensor_tensor(yo[:], yo[:], gamma_b[:], OP.mult)
                        if apply_beta:
                            nc.vector.tensor_tensor(yo[:], yo[:], beta_b[:], OP.add)
                        nc.sync.dma_start(out_d.ap()[ts, :], yo[:])

            wqp_cm.__exit__(None, None, None)

    nc.compile()
    return nc


def kernel(x, Wq, bq, Wk, bk, Wv, bv, Wo, bo, gamma, beta):
    global LAST_RESULT
    x = np.asarray(x, dtype=np.float32)
    f32 = np.float32
    bf16 = ml_dtypes.bfloat16

    apply_bias = any(np.any(np.asarray(b)) for b in (bq, bk, bv, bo))
    apply_gamma = not np.all(np.asarray(gamma) == 1.0)
    apply_beta = bool(np.any(np.asarray(beta)))

    nc = _build(apply_bias, apply_gamma, apply_beta)

    def split(W):
        W = np.asarray(W, dtype=f32)
        hi = W.astype(bf16)
        lo = (W - hi.astype(f32)).astype(bf16)
        return hi, lo

    wq_h, wq_l = split(Wq)
    wk_h, wk_l = split(Wk)
    wv_h, _ = split(Wv)
    wo_h, _ = split(Wo)
    e_sel = np.zeros((2, P), dtype=bf16)
    e_sel[0, :64] = 1
    e_sel[1, 64:] = 1

    in_maps = []
    for c in range(NCORES):
        b, half = c // 2, c % 2
        xs = x[b, half * TOK:(half + 1) * TOK]          # [2048, 1024]
        xhi = xs.astype(bf16)
        xlo = (xs - xhi.astype(f32)).astype(bf16)
        m = {
            "xthi": np.ascontiguousarray(xhi.T),
            "xtlo": np.ascontiguousarray(xlo.T),
            "wqh": wq_h, "wql": wq_l,
            "wkh": wk_h, "wkl": wk_l,
            "wvh": wv_h, "woh": wo_h,
            "xres": np.ascontiguousarray(xs),
            "e_sel": e_sel,
        }
        if apply_bias:
            m.update(bq=np.asarray(bq, f32), bk=np.asarray(bk, f32),
                     bv=np.asarray(bv, f32), bo=np.asarray(bo, f32))
        if apply_gamma:
            m["gamma"] = np.asarray(gamma, f32)
        if apply_beta:
            m["beta"] = np.asarray(beta, f32)
        in_maps.append(m)

    import os
    try:
        LAST_RESULT = run_bass_kernel_spmd(nc, in_maps, core_ids=list(range(NCORES)))
    except ModuleNotFoundError:
        # no antenv.axon_hooks in this container -> NTFF tracing unavailable
        os.environ["BASS_NEVER_TRACE"] = "1"
        LAST_RESULT = run_bass_kernel_spmd(nc, in_maps, core_ids=list(range(NCORES)))
    out = np.empty((B, N, D), dtype=np.float32)
    for c in range(NCORES):
        b, half = c // 2, c % 2
        out[b, half * TOK:(half + 1) * TOK] = LAST_RESULT.results[c]["out"]
    return out



# revision 9
# speedup vs baseline: 1.3691x; 1.3691x over previous
"""Distributed Trainium2 kernel for the linear-attention transformer block.

Math (per batch element b):
  Q = elu(x @ Wq + bq), K = elu(x @ Wk + bk), V = x @ Wv + bv   (per-head d=64)
  KV_h = K_h^T V_h  [64,64];  Ksum_h = sum_n K_h[n]  [64]
  attn_h = (Q_h @ KV_h) / (Q_h . Ksum_h)
  out = LayerNorm(x + attn @ Wo + bo) * gamma + beta

Sharding: 16384 tokens over 8 cores (2048 each; core c owns batch c//2,
half c%2). Each core computes Q/K/V only for its tokens, partial KV/Ksum,
then a 266KB AllReduce over core pairs {2b, 2b+1} completes the KV stats;
attention + output projection + LayerNorm finish locally.

Precision: Q/K/V projections and the attention matmuls run in fp32r
(single-pass; FP22 on hardware, full fp32 in the simulator) — this gives
the near-zero 1/(Q.Ksum) denominators ~1e-4 accuracy without the 3-term
bf16 split passes. The KV einsum and output projection run in bf16.
Validated ~1.9e-3 global rel err vs the f32 reference (emulating FP22).
"""

import sys

sys.path.insert(0, "/opt/trn_rl_repo")

import numpy as np
import ml_dtypes

import concourse.bass as bass
import concourse.mybir as mybir
import concourse.tile as tile
from concourse import bacc
from concourse.bass_utils import run_bass_kernel_spmd

AF = mybir.ActivationFunctionType
OP = mybir.AluOpType
F32 = mybir.dt.float32
F32R = mybir.dt.float32r
BF16 = mybir.dt.bfloat16

B, N, D = 4, 4096, 1024
H, HD = 16, 64
TOK = 2048            # tokens per core
NCORES = 8
LN_EPS = 1e-3
P = 128
KC = D // P           # 8 contraction chunks
TC = TOK // P         # 16 token chunks of 128
TQ = TOK // 512       # 4 token chunks of 512

LAST_RESULT = None    # BassKernelResults of the most recent run (for test.py)


def _build(apply_bias, apply_gamma, apply_beta):
    nc = bacc.Bacc("TRN2", target_bir_lowering=False, debug=False, num_devices=NCORES)

    def din(name, shape, dtype):
        return nc.dram_tensor(name, shape, dtype, kind="ExternalInput")

    xt = din("xt", [D, TOK], F32R)
    wq_d = din("wq", [D, D], F32R)
    wk_d = din("wk", [D, D], F32R)
    wv_d = din("wv", [D, D], F32R)
    wo_d = din("wo", [D, D], BF16)
    xres = din("xres", [TOK, D], BF16)
    e_sel = din("e_sel", [2, P], BF16)
    if apply_bias:
        bq_d = din("bq", [D], F32)
        bk_d = din("bk", [D], F32)
        bv_d = din("bv", [D], F32)
        bo_d = din("bo", [D], F32)
    if apply_gamma:
        gamma_d = din("gamma", [D], F32)
    if apply_beta:
        beta_d = din("beta", [D], F32)
    out_d = nc.dram_tensor("out", [TOK, D], F32, kind="ExternalOutput")

    r8 = lambda t: t.ap().rearrange("(ko p) n -> p ko n", p=P)

    def bcast_row(dram_vec, sb_tile):
        # DMA-broadcast a [D] vector to [P, D] (stride-0 partition dim).
        src = bass.AP(
            tensor=dram_vec.ap().tensor,
            offset=dram_vec.ap().offset,
            ap=[[0, P]] + list(dram_vec.ap().ap),
        )
        nc.sync.dma_start(out=sb_tile, in_=src)

    with tile.TileContext(nc) as tc:
        with (
            tc.tile_pool(name="xpool", bufs=1) as xpool,
            tc.tile_pool(name="smalls", bufs=1) as smalls,
            tc.tile_pool(name="dram", bufs=1, space="DRAM") as dram,
        ):
            # ---- resident x^T (fp32r) ----
            xt_sb = xpool.tile([P, KC, TOK], F32R)

            e_sb = smalls.tile([2, P], BF16)
            nc.sync.dma_start(e_sb[:], e_sel.ap())
            ones_sb = smalls.tile([P, 1], F32)
            nc.vector.memset(ones_sb[:], 1.0)
            eps_sb = smalls.tile([P, 1], F32)
            nc.vector.memset(eps_sb[:], LN_EPS)
            if apply_bias:
                bq_sb = smalls.tile([P, KC], F32)   # per-partition layout for Q^T
                nc.sync.dma_start(bq_sb[:], bq_d.ap().rearrange("(ko p) -> p ko", p=P))
                bk_b = smalls.tile([P, D], F32)
                bv_b = smalls.tile([P, D], F32)
                bo_b = smalls.tile([P, D], F32)
                bcast_row(bk_d, bk_b[:])
                bcast_row(bv_d, bv_b[:])
                bcast_row(bo_d, bo_b[:])
            if apply_gamma:
                gamma_b = smalls.tile([P, D], F32)
                bcast_row(gamma_d, gamma_b[:])
            if apply_beta:
                beta_b = smalls.tile([P, D], F32)
                bcast_row(beta_d, beta_b[:])

            # PE warmup: matmuls on zeroed tiles release the HAM clock
            # gate (1.2->2.4 GHz) while the first input DMAs are in flight.
            with (
                tc.tile_pool(name="warmsb", bufs=1) as warmsb,
                tc.tile_pool(name="warmps", bufs=2, space="PSUM") as warmps,
            ):
                warm_a = warmsb.tile([P, P], BF16)
                warm_b = warmsb.tile([P, 512], BF16)
                nc.gpsimd.memset(warm_a[:], 0.0)
                nc.gpsimd.memset(warm_b[:], 0.0)
                for w in range(64):
                    wp = warmps.tile([P, 512], F32, tag="warm", name=f"warm_{w}")
                    nc.tensor.matmul(wp[:], warm_a[:], warm_b[:], start=True, stop=True)

            # Prefetch the first two Q-weight slices; their DMAs have no deps
            # and fill otherwise-idle DMA time during phase 1.
            wqp_cm = tc.tile_pool(name="wqp", bufs=3)
            wqp = wqp_cm.__enter__()
            wq_tiles = {}

            def load_wq(hp):
                msl = slice(hp * P, (hp + 1) * P)
                wq_t = wqp.tile([P, KC, P], F32R, tag="wq", name=f"wq_{hp}")
                nc.sync.dma_start(wq_t[:], r8(wq_d)[:, :, msl])
                wq_tiles[hp] = wq_t

            # ================= Phase 1: K, V, partial KV + Ksum =================
            with (
                tc.tile_pool(name="wkv", bufs=1) as wkv,
                tc.tile_pool(name="kvps_pool", bufs=1, space="PSUM") as kvps_pool,
                tc.tile_pool(name="ph1ps", bufs=7, space="PSUM") as ph1ps,
                tc.tile_pool(name="ph1sb", bufs=4) as ph1sb,
            ):
                wk_sb = wkv.tile([P, KC, D], F32R)
                wv_sb = wkv.tile([P, KC, D], F32R)
                for k in range(KC):
                    nc.sync.dma_start(xt_sb[:, k, :], r8(xt)[:, k, :])
                    nc.sync.dma_start(wk_sb[:, k, :], r8(wk_d)[:, k, :])
                    nc.sync.dma_start(wv_sb[:, k, :], r8(wv_d)[:, k, :])
                load_wq(0)
                load_wq(1)

                # SBUF accumulator (DVE-add per token chunk; interleaved
                # multi-chunk PSUM accumulation groups proved unreliable).
                # Layout per dh half: [dh*260, dh*260+256) = KV, [+256, +260) = Ksum.
                acc = smalls.tile([P, 520], F32)
                nc.vector.memset(acc[:], 0.0)

                for t in range(TC):
                    ts = slice(t * P, (t + 1) * P)
                    kb_chunks = []
                    kvs_tiles = {}
                    for dh in range(2):
                        dsl = slice(dh * 512, (dh + 1) * 512)
                        kps = ph1ps.tile([P, 512], F32, tag="proj", name=f"kps_{t}_{dh}")
                        for k in range(KC):
                            nc.tensor.matmul(kps[:], xt_sb[:, k, ts], wk_sb[:, k, dsl],
                                             start=(k == 0), stop=(k == KC - 1))
                        if apply_bias:
                            kraw = ph1sb.tile([P, 512], F32, tag="kraw", name=f"kraw_{t}_{dh}")
                            nc.vector.tensor_tensor(kraw[:], kps[:], bk_b[:, dsl], OP.add)
                            ksrc = kraw
                        else:
                            ksrc = kps
                        kmin = ph1sb.tile([P, 512], F32, tag="kmin", name=f"kmin_{t}_{dh}")
                        nc.vector.tensor_scalar(kmin[:], ksrc[:], 0.0, None, OP.min)
                        kexp = ph1sb.tile([P, 512], F32, tag="kexp", name=f"kexp_{t}_{dh}")
                        nc.scalar.activation(kexp[:], kmin[:], AF.Exp)
                        kmax = ph1sb.tile([P, 512], F32, tag="kmax", name=f"kmax_{t}_{dh}")
                        nc.vector.tensor_scalar(kmax[:], ksrc[:], 0.0, -1.0, OP.max, OP.add)
                        kf = ph1sb.tile([P, 512], F32, tag="kf", name=f"kf_{t}_{dh}")
                        nc.vector.tensor_tensor(kf[:], kmax[:], kexp[:], OP.add)
                        kb = ph1sb.tile([P, 512], BF16, tag="kb", name=f"kb_{t}_{dh}")
                        nc.vector.tensor_copy(kb[:], kf[:])
                        kb_chunks.append(kb)
                        # Ksum column blocks (f32 matmul against ones) go into
                        # cols [256, 260) of the shared kvs_t psum tile.
                        kvs_t = kvps_pool.tile([P, 260], F32, tag="kvs_t",
                                               name=f"kvs_t_{t}_{dh}", bufs=1)
                        kvs_tiles[dh] = kvs_t
                        for j in range(4):
                            nc.tensor.matmul(
                                kvs_t[:, 256 + j:257 + j], kf[:, j * P:(j + 1) * P],
                                ones_sb[:], start=True, stop=True, skip_group_check=True)
                    for dh in range(2):
                        dsl = slice(dh * 512, (dh + 1) * 512)
                        vps = ph1ps.tile([P, 512], F32, tag="proj", name=f"vps_{t}_{dh}")
                        for k in range(KC):
                            nc.tensor.matmul(vps[:], xt_sb[:, k, ts], wv_sb[:, k, dsl],
                                             start=(k == 0), stop=(k == KC - 1))
                        vb = ph1sb.tile([P, 512], BF16, tag="vb", name=f"vb_{t}_{dh}")
                        if apply_bias:
                            nc.vector.tensor_tensor(vb[:], vps[:], bv_b[:, dsl], OP.add)
                        else:
                            nc.any.tensor_copy(vb[:], vps[:])
                        kb = kb_chunks[dh]
                        kvs_t = kvs_tiles[dh]
                        for hh in range(8):
                            h = dh * 8 + hh
                            pr = (h % 2) * 64
                            fc = (h // 2) * 64 - dh * 256
                            nc.tensor.matmul(
                                kvs_t[pr:pr + 64, fc:fc + 64],
                                kb[:, hh * 64:(hh + 1) * 64],
                                vb[:, hh * 64:(hh + 1) * 64],
                                start=True, stop=True,
                                tile_position=(0, pr), skip_group_check=True)
                        nc.vector.tensor_tensor(
                            acc[:, dh * 260:(dh + 1) * 260],
                            acc[:, dh * 260:(dh + 1) * 260], kvs_t[:], OP.add)

            # ========== Phases 2-4: AllReduce; Q^T; attention (pipelined) ==========
            with tc.tile_pool(name="late", bufs=1) as late:
                at_sb = late.tile([P, KC, TOK], BF16)
                wo_sb = late.tile([P, KC, D], BF16)

                with (
                    tc.tile_pool(name="qtp", bufs=4) as qtp,
                    tc.tile_pool(name="ph3ps", bufs=3, space="PSUM") as ph3ps,
                    tc.tile_pool(name="ph3sb", bufs=3) as ph3sb,
                    tc.tile_pool(name="ph4ps_d", bufs=1, space="PSUM") as ph4ps_d,
                    tc.tile_pool(name="ph4ps_z", bufs=2, space="PSUM") as ph4ps_z,
                    tc.tile_pool(name="ph4ps_a", bufs=2, space="PSUM") as ph4ps_a,
                    tc.tile_pool(name="ph4sb", bufs=4) as ph4sb,
                ):
                    qt_tiles = {}

                    # -- AllReduce of the packed KV/Ksum accumulator --
                    cc_in = dram.tile([P, 520], F32)
                    cc_out = dram.tile([P, 520], F32)
                    nc.sync.dma_start(cc_in[:], acc[:])
                    nc.gpsimd.collective_compute(
                        "AllReduce", OP.add,
                        replica_groups=[[0, 1], [2, 3], [4, 5], [6, 7]],
                        ins=[cc_in[:].opt()], outs=[cc_out[:].opt()])
                    ar_sb = smalls.tile([P, 520], F32)
                    nc.sync.dma_start(ar_sb[:], cc_out[:])
                    kv_bf = smalls.tile([P, 512], BF16)
                    nc.gpsimd.tensor_copy(kv_bf[:, :256], ar_sb[:, :256])
                    nc.gpsimd.tensor_copy(kv_bf[:, 256:], ar_sb[:, 260:516])
                    kd_sb = smalls.tile([P, H], F32)  # col h: Ksum_h at rows (h%2)*64
                    nc.gpsimd.memset(kd_sb[:], 0.0)
                    for h in range(H):
                        pr = (h % 2) * 64
                        c = h // 2
                        sc = 256 + c if c < 4 else 516 + (c - 4)
                        nc.gpsimd.tensor_copy(
                            kd_sb[pr:pr + 64, h:h + 1], ar_sb[pr:pr + 64, sc:sc + 1])

                    for k in range(KC):
                        nc.sync.dma_start(wo_sb[:, k, :], r8(wo_d)[:, k, :])

                    def q_proj(hp):
                        wq_t = wq_tiles.pop(hp)
                        qt = qtp.tile([P, TOK], F32, tag="qt", name=f"qt_{hp}")
                        qt_tiles[hp] = qt
                        for tq in range(TQ):
                            tsl = slice(tq * 512, (tq + 1) * 512)
                            qps = ph3ps.tile([P, 512], F32, tag="qps", name=f"qps_{hp}_{tq}")
                            for k in range(KC):
                                nc.tensor.matmul(qps[:], wq_t[:, k, :], xt_sb[:, k, tsl],
                                                 start=(k == 0), stop=(k == KC - 1))
                            if apply_bias:
                                qraw = ph3sb.tile([P, 512], F32, tag="qraw", name=f"qraw_{hp}_{tq}")
                                nc.vector.tensor_scalar(qraw[:], qps[:], bq_sb[:, hp:hp + 1],
                                                        None, OP.add)
                                qsrc = qraw
                            else:
                                qsrc = qps
                            qmin = ph3sb.tile([P, 512], F32, tag="qmin", name=f"qmin_{hp}_{tq}")
                            nc.vector.tensor_scalar(qmin[:], qsrc[:], 0.0, None, OP.min)
                            qexp = ph3sb.tile([P, 512], F32, tag="qexp", name=f"qexp_{hp}_{tq}")
                            nc.scalar.activation(qexp[:], qmin[:], AF.Exp)
                            qmax = ph3sb.tile([P, 512], F32, tag="qmax", name=f"qmax_{hp}_{tq}")
                            nc.vector.tensor_scalar(qmax[:], qsrc[:], 0.0, -1.0, OP.max, OP.add)
                            nc.vector.tensor_tensor(qt[:, tsl], qmax[:], qexp[:], OP.add)

                    def attention(hp):
                        qt = qt_tiles.pop(hp)
                        for tq in range(TQ):
                            tsl = slice(tq * 512, (tq + 1) * 512)
                            dps = ph4ps_d.tile([2, 512], F32, tag="dps", name=f"dps_{hp}_{tq}")
                            nc.tensor.matmul(dps[:], kd_sb[:, 2 * hp:2 * hp + 2],
                                             qt[:, tsl], start=True, stop=True)
                            zrf = ph4sb.tile([2, 512], F32, tag="zrf", name=f"zrf_{hp}_{tq}")
                            nc.vector.reciprocal(zrf[:], dps[:])
                            zr = ph4sb.tile([2, 512], BF16, tag="zr", name=f"zr_{hp}_{tq}")
                            nc.vector.tensor_copy(zr[:], zrf[:])
                            zps = ph4ps_z.tile([P, 512], F32, tag="zps", name=f"zps_{hp}_{tq}")
                            nc.tensor.matmul(zps[:], e_sb[:], zr[:], start=True, stop=True)
                            zf = ph4sb.tile([P, 512], F32, tag="zf", name=f"zf_{hp}_{tq}")
                            nc.vector.tensor_copy(zf[:], zps[:])
                            qbf = ph4sb.tile([P, 512], BF16, tag="qbf", name=f"qbf_{hp}_{tq}")
                            nc.vector.tensor_copy(qbf[:], qt[:, tsl])
                            aps = ph4ps_a.tile([P, 512], F32, tag="aps", name=f"aps_{hp}_{tq}")
                            for par in (0, 64):
                                nc.tensor.matmul(
                                    aps[par:par + 64, :],
                                    kv_bf[par:par + 64, hp * 64:(hp + 1) * 64],
                                    qbf[par:par + 64, :],
                                    start=True, stop=True, tile_position=(par, par))
                            nc.vector.tensor_tensor(at_sb[:, hp, tsl], aps[:], zf[:], OP.mult)

                    # depth-2 software pipeline: attention(hp) runs two Q chunks
                    # behind, so the AllReduce hides under ~3 Q projections.
                    q_proj(0)
                    for hp in range(1, KC):
                        if hp + 1 < KC:
                            load_wq(hp + 1)
                        q_proj(hp)
                        if hp >= 2:
                            attention(hp - 2)
                    attention(KC - 2)
                    attention(KC - 1)

                # ===== Phase 5: output projection + residual + LayerNorm =====
                with (
                    tc.tile_pool(name="ph5ps", bufs=3, space="PSUM") as ph5ps,
                    tc.tile_pool(name="ph5sb", bufs=3) as ph5sb,
                ):
                    for t in range(TC):
                        ts = slice(t * P, (t + 1) * P)
                        y = ph5sb.tile([P, D], F32, tag="y", name=f"y_{t}")
                        xr = ph5sb.tile([P, D], BF16, tag="xr", name=f"xr_{t}")
                        nc.sync.dma_start(xr[:], xres.ap()[ts, :])
                        ops = ph5ps.tile([P, D], F32, tag="ops", name=f"ops_{t}")
                        for dh in range(2):
                            dsl = slice(dh * 512, (dh + 1) * 512)
                            for c in range(KC):
                                nc.tensor.matmul(ops[:, dsl], at_sb[:, c, ts], wo_sb[:, c, dsl],
                                                 start=(c == 0), stop=(c == KC - 1))
                        nc.vector.tensor_tensor(y[:], ops[:], xr[:], OP.add)
                        if apply_bias:
                            nc.vector.tensor_tensor(y[:], y[:], bo_b[:], OP.add)
                        stats = ph5sb.tile([P, 2, 6], F32, tag="stats", name=f"stats_{t}")
                        nc.vector.bn_stats(out=stats[:, 0, :], in_=y[:, :512])
                        nc.vector.bn_stats(out=stats[:, 1, :], in_=y[:, 512:])
                        mv = ph5sb.tile([P, 2], F32, tag="mv", name=f"mv_{t}")
                        nc.vector.bn_aggr(out=mv[:], in_=stats[:])
                        nc.scalar.activation(out=mv[:, 1:2], in_=mv[:, 1:2], func=AF.Sqrt,
                                             bias=eps_sb[:], scale=1.0)
                        nc.vector.reciprocal(mv[:, 1:2], mv[:, 1:2])
                        yo = ph5sb.tile([P, D], F32, tag="yo", name=f"yo_{t}")
                        nc.gpsimd.tensor_scalar(yo[:], y[:], mv[:, 0:1], mv[:, 1:2],
                                                OP.subtract, OP.mult)
                        if apply_gamma:
                            nc.vector.tensor_tensor(yo[:], yo[:], gamma_b[:], OP.mult)
                        if apply_beta:
                            nc.vector.tensor_tensor(yo[:], yo[:], beta_b[:], OP.add)
                        nc.sync.dma_start(out_d.ap()[ts, :], yo[:])

            wqp_cm.__exit__(None, None, None)

    nc.compile()
    return nc


def kernel(x, Wq, bq, Wk, bk, Wv, bv, Wo, bo, gamma, beta):
    global LAST_RESULT
    x = np.asarray(x, dtype=np.float32)
    f32 = np.float32
    bf16 = ml_dtypes.bfloat16

    apply_bias = any(np.any(np.asarray(b)) for b in (bq, bk, bv, bo))
    apply_gamma = not np.all(np.asarray(gamma) == 1.0)
    apply_beta = bool(np.any(np.asarray(beta)))

    nc = _build(apply_bias, apply_gamma, apply_beta)

    wq_f = np.asarray(Wq, f32)
    wk_f = np.asarray(Wk, f32)
    wv_f = np.asarray(Wv, f32)
    wo_b = np.asarray(Wo, f32).astype(bf16)
    e_sel = np.zeros((2, P), dtype=bf16)
    e_sel[0, :64] = 1
    e_sel[1, 64:] = 1

    in_maps = []
    for c in range(NCORES):
        b, half = c // 2, c % 2
        xs = x[b, half * TOK:(half + 1) * TOK]          # [2048, 1024]
        m = {
            "xt": np.ascontiguousarray(xs.T),
            "wq": wq_f, "wk": wk_f, "wv": wv_f, "wo": wo_b,
            "xres": xs.astype(bf16),
            "e_sel": e_sel,
        }
        if apply_bias:
            m.update(bq=np.asarray(bq, f32), bk=np.asarray(bk, f32),
                     bv=np.asarray(bv, f32), bo=np.asarray(bo, f32))
        if apply_gamma:
            m["gamma"] = np.asarray(gamma, f32)
        if apply_beta:
            m["beta"] = np.asarray(beta, f32)
        in_maps.append(m)

    import os
    try:
        LAST_RESULT = run_bass_kernel_spmd(nc, in_maps, core_ids=list(range(NCORES)))
    except ModuleNotFoundError:
        # no antenv.axon_hooks in this container -> NTFF tracing unavailable
        os.environ["BASS_NEVER_TRACE"] = "1"
        LAST_RESULT = run_bass_kernel_spmd(nc, in_maps, core_ids=list(range(NCORES)))
    out = np.empty((B, N, D), dtype=np.float32)
    for c in range(NCORES):
        b, half = c // 2, c % 2
        out[b, half * TOK:(half + 1) * TOK] = LAST_RESULT.results[c]["out"]
    return out


# revision 64
# speedup vs baseline: 1.3947x; 1.0186x over previous
"""Distributed Trainium2 kernel for the linear-attention transformer block.

Math (per batch element b):
  Q = elu(x @ Wq + bq), K = elu(x @ Wk + bk), V = x @ Wv + bv   (per-head d=64)
  KV_h = K_h^T V_h  [64,64];  Ksum_h = sum_n K_h[n]  [64]
  attn_h = (Q_h @ KV_h) / (Q_h . Ksum_h)
  out = LayerNorm(x + attn @ Wo + bo) * gamma + beta

Sharding: 16384 tokens over 8 cores (2048 each; core c owns batch c//2,
half c%2). Each core computes Q/K/V only for its tokens, partial KV/Ksum,
then a 266KB AllReduce over core pairs {2b, 2b+1} completes the KV stats;
attention + output projection + LayerNorm finish locally.

Precision: Q/K/V projections and the attention matmuls run in fp32r
(single-pass; FP22 on hardware, full fp32 in the simulator) — this gives
the near-zero 1/(Q.Ksum) denominators ~1e-4 accuracy without the 3-term
bf16 split passes. The KV einsum and output projection run in bf16.
Validated ~1.9e-3 global rel err vs the f32 reference (emulating FP22).
"""

import sys

sys.path.insert(0, "/opt/trn_rl_repo")

import numpy as np
import ml_dtypes

import concourse.bass as bass
import concourse.mybir as mybir
import concourse.tile as tile
from concourse import bacc
from concourse.bass_utils import run_bass_kernel_spmd

AF = mybir.ActivationFunctionType
OP = mybir.AluOpType
F32 = mybir.dt.float32
F32R = mybir.dt.float32r
BF16 = mybir.dt.bfloat16
FP8 = mybir.dt.float8e4
DR = mybir.MatmulPerfMode.DoubleRow

B, N, D = 4, 4096, 1024
H, HD = 16, 64
TOK = 2048            # tokens per core
NCORES = 8
LN_EPS = 1e-3
P = 128
KC = D // P           # 8 contraction chunks
TC = TOK // P         # 16 token chunks of 128
TQ = TOK // 512       # 4 token chunks of 512

LAST_RESULT = None    # BassKernelResults of the most recent run (for test.py)


def _build(apply_bias, apply_gamma, apply_beta):
    nc = bacc.Bacc("TRN2", target_bir_lowering=False, debug=False, num_devices=NCORES)

    def din(name, shape, dtype):
        return nc.dram_tensor(name, shape, dtype, kind="ExternalInput")

    xt = din("xt", [D, TOK], F32R)
    wq_d = din("wq", [D, D], F32R)
    wk_d = din("wk", [D, D], F32R)
    x8_d = din("x8", [D, 2, TOK], FP8)    # levels: [.,0,.]=fp8(x), [.,1,.]=fp8(x-h)
    xl8_d = din("xl8", [D, TOK], FP8)     # fp8((x - rne11(x)) * 4096)
    wq8_d = din("wq8", [D, 2, D], FP8)    # [.,0,.]=fp8(rne11(wq)), [.,1,.]=fp8((wq-rne11(wq))*4096)
    wv8_d = din("wv8", [D, 2, D], FP8)
    wo_d = din("wo", [D, D], BF16)
    xres = din("xres", [TOK, D], BF16)
    e_sel = din("e_sel", [2, P], BF16)
    if apply_bias:
        bq_d = din("bq", [D], F32)
        bk_d = din("bk", [D], F32)
        bv_d = din("bv", [D], F32)
        bo_d = din("bo", [D], F32)
    if apply_gamma:
        gamma_d = din("gamma", [D], F32)
    if apply_beta:
        beta_d = din("beta", [D], F32)
    out_d = nc.dram_tensor("out", [TOK, D], F32, kind="ExternalOutput")

    r8 = lambda t: t.ap().rearrange("(ko p) n -> p ko n", p=P)

    def bcast_row(dram_vec, sb_tile):
        # DMA-broadcast a [D] vector to [P, D] (stride-0 partition dim).
        src = bass.AP(
            tensor=dram_vec.ap().tensor,
            offset=dram_vec.ap().offset,
            ap=[[0, P]] + list(dram_vec.ap().ap),
        )
        nc.sync.dma_start(out=sb_tile, in_=src)

    with tile.TileContext(nc) as tc:
        with (
            tc.tile_pool(name="smalls", bufs=1) as smalls,
            tc.tile_pool(name="dram", bufs=1, space="DRAM") as dram,
        ):
            # ---- resident x^T (fp32r m11 main + fp8 Q-correction levels) ----
            xpool_cm = tc.tile_pool(name="xpool", bufs=1)
            xpool = xpool_cm.__enter__()
            xt_sb = xpool.tile([P, KC, TOK], F32R)
            x8h_sb = xpool.tile([P, KC, TOK], FP8)
            xl8_sb = xpool.tile([P, KC, TOK], FP8)

            e_sb = smalls.tile([2, P], BF16)
            nc.sync.dma_start(e_sb[:], e_sel.ap())
            ones_sb = smalls.tile([P, 1], F32)
            nc.vector.memset(ones_sb[:], 1.0)
            eps_sb = smalls.tile([P, 1], F32)
            nc.vector.memset(eps_sb[:], LN_EPS)
            if apply_bias:
                bq_sb = smalls.tile([P, KC], F32)   # per-partition layout for Q^T
                nc.sync.dma_start(bq_sb[:], bq_d.ap().rearrange("(ko p) -> p ko", p=P))
                bk_b = smalls.tile([P, D], F32)
                bv_b = smalls.tile([P, D], F32)
                bo_b = smalls.tile([P, D], F32)
                bcast_row(bk_d, bk_b[:])
                bcast_row(bv_d, bv_b[:])
                bcast_row(bo_d, bo_b[:])
            if apply_gamma:
                gamma_b = smalls.tile([P, D], F32)
                bcast_row(gamma_d, gamma_b[:])
            if apply_beta:
                beta_b = smalls.tile([P, D], F32)
                bcast_row(beta_d, beta_b[:])

            # PE warmup: matmuls on zeroed tiles release the HAM clock
            # gate (1.2->2.4 GHz) while the first input DMAs are in flight.
            with (
                tc.tile_pool(name="warmsb", bufs=1) as warmsb,
                tc.tile_pool(name="warmps", bufs=2, space="PSUM") as warmps,
            ):
                warm_a = warmsb.tile([P, P], BF16)
                warm_b = warmsb.tile([P, 512], BF16)
                nc.gpsimd.memset(warm_a[:], 0.0)
                nc.gpsimd.memset(warm_b[:], 0.0)
                for w in range(40):
                    wp = warmps.tile([P, 512], F32, tag="warm", name=f"warm_{w}")
                    nc.tensor.matmul(wp[:], warm_a[:], warm_b[:], start=True, stop=True)

            # Prefetch the first two Q-weight slices; their DMAs have no deps
            # and fill otherwise-idle DMA time during phase 1.
            r82g = lambda t: t.ap().rearrange("(ko p) two n -> p ko two n", p=P)
            wqp_cm = tc.tile_pool(name="wqp", bufs=2)
            wqp = wqp_cm.__enter__()
            wq_tiles = {}

            def load_wq(hp):
                msl = slice(hp * P, (hp + 1) * P)
                wq_t = wqp.tile([P, KC, P], F32R, tag="wq", name=f"wq_{hp}")
                nc.sync.dma_start(wq_t[:], r8(wq_d)[:, :, msl])
                wq8_t = wqp.tile([P, KC, 2, P], FP8, tag="wq8", name=f"wq8_{hp}")
                for lv in range(2):
                    nc.sync.dma_start(wq8_t[:, :, lv, :], r82g(wq8_d)[:, :, lv, msl])
                wq_tiles[hp] = (wq_t, wq8_t)

            # ================= Phase 1: K, V, partial KV + Ksum =================
            with (
                tc.tile_pool(name="wkv", bufs=1) as wkv,
                tc.tile_pool(name="kvps_pool", bufs=1, space="PSUM") as kvps_pool,
                tc.tile_pool(name="ph1ps", bufs=5, space="PSUM") as ph1ps,
                tc.tile_pool(name="ph1sb", bufs=2) as ph1sb,
            ):
                wk_sb = wkv.tile([P, KC, D], F32R)
                wv8_sb = wkv.tile([P, KC, 2, D], FP8)
                x8m_sb = wkv.tile([P, KC, TOK], FP8)
                # DMA priority order: weights and the first token slices
                # first, then token-sliced round-robin so K (xt) stays ahead
                # of V (x8) as the t-loop advances.
                for k in range(KC):
                    nc.sync.dma_start(wk_sb[:, k, :], r8(wk_d)[:, k, :])
                    nc.sync.dma_start(xt_sb[:, k, :256], r8(xt)[:, k, :256])
                for k in range(KC):
                    nc.sync.dma_start(wv8_sb[:, k, :, :], r82g(wv8_d)[:, k, :, :])
                    nc.sync.dma_start(x8h_sb[:, k, :256], r82g(x8_d)[:, k, 0, :256])
                    nc.sync.dma_start(x8m_sb[:, k, :256], r82g(x8_d)[:, k, 1, :256])
                for lo, hi in ((256, 768), (768, 1280), (1280, 1792), (1792, 2048)):
                    sl = slice(lo, hi)
                    for k in range(KC):
                        nc.sync.dma_start(xt_sb[:, k, sl], r8(xt)[:, k, sl])
                    for k in range(KC):
                        nc.sync.dma_start(x8h_sb[:, k, sl], r82g(x8_d)[:, k, 0, sl])
                        nc.sync.dma_start(x8m_sb[:, k, sl], r82g(x8_d)[:, k, 1, sl])
                for k in range(KC):
                    nc.sync.dma_start(xl8_sb[:, k, :], r8(xl8_d)[:, k, :])
                load_wq(0)

                # SBUF accumulator (DVE-add per token chunk; interleaved
                # multi-chunk PSUM accumulation groups proved unreliable).
                # Layout per dh half: [dh*260, dh*260+256) = KV, [+256, +260) = Ksum.
                acc = smalls.tile([P, 520], F32)
                nc.vector.memset(acc[:], 0.0)

                for t in range(TC):
                    ts = slice(t * P, (t + 1) * P)
                    kb_chunks = []
                    kvs_tiles = {}
                    for dh in range(2):
                        dsl = slice(dh * 512, (dh + 1) * 512)
                        kps = ph1ps.tile([P, 512], F32, tag="proj", name=f"kps_{t}_{dh}")
                        for k in range(KC):
                            nc.tensor.matmul(kps[:], xt_sb[:, k, ts], wk_sb[:, k, dsl],
                                             start=(k == 0), stop=(k == KC - 1))
                        if apply_bias:
                            kraw = ph1sb.tile([P, 512], F32, tag="kraw", name=f"kraw_{t}_{dh}")
                            nc.vector.tensor_tensor(kraw[:], kps[:], bk_b[:, dsl], OP.add)
                            ksrc = kraw
                        else:
                            ksrc = kps
                        kmin = ph1sb.tile([P, 512], F32, tag="kmin", name=f"kmin_{t}_{dh}")
                        nc.vector.tensor_scalar(kmin[:], ksrc[:], 0.0, None, OP.min)
                        kexp = ph1sb.tile([P, 512], F32, tag="kexp", name=f"kexp_{t}_{dh}")
                        nc.scalar.activation(kexp[:], kmin[:], AF.Exp)
                        kmax = ph1sb.tile([P, 512], F32, tag="kmax", name=f"kmax_{t}_{dh}")
                        nc.vector.tensor_scalar(kmax[:], ksrc[:], 0.0, -1.0, OP.max, OP.add)
                        kf = ph1sb.tile([P, 512], F32, tag="kf", name=f"kf_{t}_{dh}")
                        nc.vector.tensor_tensor(kf[:], kmax[:], kexp[:], OP.add)
                        kb = ph1sb.tile([P, 512], BF16, tag="kb", name=f"kb_{t}_{dh}")
                        nc.gpsimd.tensor_copy(kb[:], kf[:])
                        kb_chunks.append(kb)
                        # Ksum column blocks (f32 matmul against ones) go into
                        # cols [256, 260) of the shared kvs_t psum tile.
                        kvs_t = kvps_pool.tile([P, 260], F32, tag="kvs_t",
                                               name=f"kvs_t_{t}_{dh}", bufs=1)
                        kvs_tiles[dh] = kvs_t
                        for j in range(4):
                            nc.tensor.matmul(
                                kvs_t[:, 256 + j:257 + j], kf[:, j * P:(j + 1) * P],
                                ones_sb[:], start=True, stop=True, skip_group_check=True)
                    for dh in range(2):
                        dsl = slice(dh * 512, (dh + 1) * 512)
                        vps = ph1ps.tile([P, 512], F32, tag="proj", name=f"vps_{t}_{dh}")
                        for lv, xlv in enumerate((x8h_sb, x8m_sb)):
                            for j in range(4):
                                nc.tensor.matmul(
                                    vps[:], xlv[:, 2 * j:2 * j + 2, ts],
                                    wv8_sb[:, 2 * j:2 * j + 2, 0, dsl],
                                    start=(lv == 0 and j == 0), stop=(lv == 1 and j == 3),
                                    perf_mode=DR)
                        vpsB = ph1ps.tile([P, 512], F32, tag="projB", name=f"vpsB_{t}_{dh}", bufs=2)
                        for j in range(4):
                            nc.tensor.matmul(
                                vpsB[:], x8h_sb[:, 2 * j:2 * j + 2, ts],
                                wv8_sb[:, 2 * j:2 * j + 2, 1, dsl],
                                start=(j == 0), stop=(j == 3), perf_mode=DR)
                        vbs = ph1sb.tile([P, 512], F32, tag="vbs", name=f"vbs_{t}_{dh}")
                        nc.scalar.activation(vbs[:], vpsB[:], AF.Copy, scale=2.0 ** -4)
                        vb = ph1sb.tile([P, 512], BF16, tag="vb", name=f"vb_{t}_{dh}")
                        nc.vector.tensor_tensor(vb[:], vps[:], vbs[:], OP.add)
                        if apply_bias:
                            nc.vector.tensor_tensor(vb[:], vb[:], bv_b[:, dsl], OP.add)
                        kb = kb_chunks[dh]
                        kvs_t = kvs_tiles[dh]
                        for hh in range(8):
                            h = dh * 8 + hh
                            pr = (h % 2) * 64
                            fc = (h // 2) * 64 - dh * 256
                            nc.tensor.matmul(
                                kvs_t[pr:pr + 64, fc:fc + 64],
                                kb[:, hh * 64:(hh + 1) * 64],
                                vb[:, hh * 64:(hh + 1) * 64],
                                start=True, stop=True,
                                tile_position=(0, pr), skip_group_check=True)
                        nc.vector.tensor_tensor(
                            acc[:, dh * 260:(dh + 1) * 260],
                            acc[:, dh * 260:(dh + 1) * 260], kvs_t[:], OP.add)

            # ========== Phases 2-4: AllReduce; Q^T; attention (pipelined) ==========
            with tc.tile_pool(name="late", bufs=1) as late:
                at_sb = late.tile([P, KC, TOK], BF16)
                qtp_cm = tc.tile_pool(name="qtp", bufs=5)
                qtp = qtp_cm.__enter__()
                qt_tiles = {}

                # -- AllReduce of the packed KV/Ksum accumulator --
                cc_in = dram.tile([P, 520], F32)
                cc_out = dram.tile([P, 520], F32)
                nc.sync.dma_start(cc_in[:], acc[:])
                nc.gpsimd.collective_compute(
                    "AllReduce", OP.add,
                    replica_groups=[[0, 1], [2, 3], [4, 5], [6, 7]],
                    ins=[cc_in[:].opt()], outs=[cc_out[:].opt()])
                ar_sb = smalls.tile([P, 520], F32)
                nc.sync.dma_start(ar_sb[:], cc_out[:])
                kv_bf = smalls.tile([P, 512], BF16)
                nc.gpsimd.tensor_copy(kv_bf[:, :256], ar_sb[:, :256])
                nc.gpsimd.tensor_copy(kv_bf[:, 256:], ar_sb[:, 260:516])
                kd_sb = smalls.tile([P, H], F32)  # col h: Ksum_h at rows (h%2)*64
                nc.gpsimd.memset(kd_sb[:], 0.0)
                for h in range(H):
                    pr = (h % 2) * 64
                    c = h // 2
                    sc = 256 + c if c < 4 else 516 + (c - 4)
                    nc.gpsimd.tensor_copy(
                        kd_sb[pr:pr + 64, h:h + 1], ar_sb[pr:pr + 64, sc:sc + 1])
                # Ksum vector in fp32r for the fp32r denominator matmul.
                kd_r = smalls.tile([P, H], F32R)
                nc.gpsimd.tensor_copy(kd_r[:], kd_sb[:])

                with (
                    tc.tile_pool(name="ph3ps", bufs=3, space="PSUM") as ph3ps,
                    tc.tile_pool(name="ph3sb", bufs=2) as ph3sb,
                    tc.tile_pool(name="ph4ps_d", bufs=1, space="PSUM") as ph4ps_d,
                    tc.tile_pool(name="ph4ps_z", bufs=1, space="PSUM") as ph4ps_z,
                    tc.tile_pool(name="ph4ps_a", bufs=2, space="PSUM") as ph4ps_a,
                    tc.tile_pool(name="ph4sb", bufs=3) as ph4sb,
                ):
                    def attention(hp):
                        qt = qt_tiles.pop(hp)
                        for tq in range(TQ):
                            tsl = slice(tq * 512, (tq + 1) * 512)
                            dps = ph4ps_d.tile([2, 512], F32, tag="dps", name=f"dps_{hp}_{tq}")
                            nc.tensor.matmul(dps[:], kd_r[:, 2 * hp:2 * hp + 2],
                                             qt[:, tsl], start=True, stop=True)
                            zr = ph4sb.tile([2, 512], BF16, tag="zr", name=f"zr_{hp}_{tq}")
                            with nc.allow_low_precision(reason="z is bf16 downstream"):
                                nc.vector.reciprocal(zr[:], dps[:])
                            zps = ph4ps_z.tile([P, 512], F32, tag="zps", name=f"zps_{hp}_{tq}")
                            nc.tensor.matmul(zps[:], e_sb[:], zr[:], start=True, stop=True)
                            zf = ph4sb.tile([P, 512], BF16, tag="zf", name=f"zf_{hp}_{tq}")
                            nc.scalar.activation(zf[:], zps[:], AF.Copy)
                            qbf = ph4sb.tile([P, 512], BF16, tag="qbf", name=f"qbf_{hp}_{tq}")
                            nc.gpsimd.tensor_copy(qbf[:], qt[:, tsl])
                            aps = ph4ps_a.tile([P, 512], F32, tag="aps", name=f"aps_{hp}_{tq}")
                            for par in (0, 64):
                                nc.tensor.matmul(
                                    aps[par:par + 64, :],
                                    kv_bf[par:par + 64, hp * 64:(hp + 1) * 64],
                                    qbf[par:par + 64, :],
                                    start=True, stop=True, tile_position=(par, par))
                            nc.vector.tensor_tensor(at_sb[:, hp, tsl], aps[:], zf[:], OP.mult)

                    def q_proj(hp):
                        wq_t, wq8_t = wq_tiles.pop(hp)
                        qt = qtp.tile([P, TOK], F32R, tag="qt", name=f"qt_{hp}")
                        qt_tiles[hp] = qt
                        for tq in range(TQ):
                            tsl = slice(tq * 512, (tq + 1) * 512)
                            # main pass: fp32r (m11 reads of x and wq)
                            qps = ph3ps.tile([P, 512], F32, tag="qps", name=f"qps_{hp}_{tq}")
                            for k in range(KC):
                                nc.tensor.matmul(qps[:], wq_t[:, k, :], xt_sb[:, k, tsl],
                                                 start=(k == 0), stop=(k == KC - 1))
                            # fp8 DoubleRow correction: the m11 residuals of x
                            # and wq, both scaled by 2^12.
                            qpsC = ph3ps.tile([P, 512], F32, tag="qpsC",
                                              name=f"qpsC_{hp}_{tq}", bufs=1)
                            for j in range(4):
                                nc.tensor.matmul(
                                    qpsC[:], wq8_t[:, 2 * j:2 * j + 2, 0, :],
                                    xl8_sb[:, 2 * j:2 * j + 2, tsl],
                                    start=(j == 0), stop=False, perf_mode=DR)
                            for j in range(4):
                                nc.tensor.matmul(
                                    qpsC[:], wq8_t[:, 2 * j:2 * j + 2, 1, :],
                                    x8h_sb[:, 2 * j:2 * j + 2, tsl],
                                    start=False, stop=(j == 3), perf_mode=DR)
                            qraw = ph3sb.tile([P, 512], F32, tag="qraw", name=f"qraw_{hp}_{tq}")
                            nc.scalar.activation(qraw[:], qpsC[:], AF.Copy, scale=2.0 ** -12)
                            nc.vector.tensor_tensor(qraw[:], qraw[:], qps[:], OP.add)
                            if apply_bias:
                                nc.vector.tensor_scalar(qraw[:], qraw[:], bq_sb[:, hp:hp + 1],
                                                        None, OP.add)
                            qsrc = qraw
                            qmin = ph3sb.tile([P, 512], F32, tag="qmin", name=f"qmin_{hp}_{tq}")
                            nc.gpsimd.tensor_scalar(qmin[:], qsrc[:], 0.0, None, OP.min)
                            qexp = ph3sb.tile([P, 512], F32, tag="qexp", name=f"qexp_{hp}_{tq}")
                            nc.scalar.activation(qexp[:], qmin[:], AF.Exp)
                            with nc.allow_low_precision(reason="f32r is f32-width"):
                                nc.vector.tensor_scalar(qt[:, tsl], qsrc[:], 0.0, -1.0,
                                                        OP.max, OP.add)
                                nc.vector.tensor_tensor(qt[:, tsl], qt[:, tsl], qexp[:], OP.add)

                    # Depth-5 software pipeline: attention(hp) runs five Q
                    # chunks behind, so the ~40us AllReduce hides under the
                    # first five (main+correction) Q projections.
                    # Depth-5 software pipeline: attention(hp) runs five Q
                    # chunks behind, so the ~40us AllReduce hides under the
                    # first five (main+correction) Q projections.
                    for hp in range(KC):
                        if hp + 1 < KC:
                            load_wq(hp + 1)
                        if hp >= 5:
                            attention(hp - 5)
                        q_proj(hp)
                    for hp in range(KC - 5, KC):
                        attention(hp)

                qtp_cm.__exit__(None, None, None)

                # ===== Phase 5: output projection + residual + LayerNorm =====
                with (
                    tc.tile_pool(name="ph5w", bufs=1) as ph5w,
                    tc.tile_pool(name="ph5ps", bufs=4, space="PSUM") as ph5ps,
                    tc.tile_pool(name="ph5sb", bufs=4) as ph5sb,
                ):
                    wo_sb = ph5w.tile([P, KC, D], BF16)
                    for k in range(KC):
                        nc.sync.dma_start(wo_sb[:, k, :], r8(wo_d)[:, k, :])
                    for t in range(TC):
                        ts = slice(t * P, (t + 1) * P)
                        y = ph5sb.tile([P, D], F32, tag="y", name=f"y_{t}")
                        xr = ph5sb.tile([P, D], BF16, tag="xr", name=f"xr_{t}")
                        nc.sync.dma_start(xr[:], xres.ap()[ts, :])
                        ops = ph5ps.tile([P, D], F32, tag="ops", name=f"ops_{t}")
                        for dh in range(2):
                            dsl = slice(dh * 512, (dh + 1) * 512)
                            for c in range(KC):
                                nc.tensor.matmul(ops[:, dsl], at_sb[:, c, ts], wo_sb[:, c, dsl],
                                                 start=(c == 0), stop=(c == KC - 1))
                        nc.vector.tensor_tensor(y[:], ops[:], xr[:], OP.add)
                        if apply_bias:
                            nc.vector.tensor_tensor(y[:], y[:], bo_b[:], OP.add)
                        stats = ph5sb.tile([P, 2, 6], F32, tag="stats", name=f"stats_{t}")
                        nc.vector.bn_stats(out=stats[:, 0, :], in_=y[:, :512])
                        nc.vector.bn_stats(out=stats[:, 1, :], in_=y[:, 512:])
                        mv = ph5sb.tile([P, 2], F32, tag="mv", name=f"mv_{t}")
                        nc.vector.bn_aggr(out=mv[:], in_=stats[:])
                        nc.scalar.activation(out=mv[:, 1:2], in_=mv[:, 1:2], func=AF.Sqrt,
                                             bias=eps_sb[:], scale=1.0)
                        nc.vector.reciprocal(mv[:, 1:2], mv[:, 1:2])
                        yo = ph5sb.tile([P, D], F32, tag="yo", name=f"yo_{t}")
                        nc.gpsimd.tensor_scalar(yo[:], y[:], mv[:, 0:1], mv[:, 1:2],
                                                OP.subtract, OP.mult)
                        if apply_gamma:
                            nc.vector.tensor_tensor(yo[:], yo[:], gamma_b[:], OP.mult)
                        if apply_beta:
                            nc.vector.tensor_tensor(yo[:], yo[:], beta_b[:], OP.add)
                        nc.sync.dma_start(out_d.ap()[ts, :], yo[:])

            wqp_cm.__exit__(None, None, None)
            xpool_cm.__exit__(None, None, None)

    nc.compile()
    return nc


def kernel(x, Wq, bq, Wk, bk, Wv, bv, Wo, bo, gamma, beta):
    global LAST_RESULT
    x = np.asarray(x, dtype=np.float32)
    f32 = np.float32
    bf16 = ml_dtypes.bfloat16

    apply_bias = any(np.any(np.asarray(b)) for b in (bq, bk, bv, bo))
    apply_gamma = not np.all(np.asarray(gamma) == 1.0)
    apply_beta = bool(np.any(np.asarray(beta)))

    nc = _build(apply_bias, apply_gamma, apply_beta)

    fp8 = ml_dtypes.float8_e4m3

    def rne11(a):
        # round-to-nearest-even to 11 mantissa bits: what the PE's fp32r
        # datapath keeps of each operand.
        u = np.ascontiguousarray(a, np.float32).view(np.uint32)
        drop = 12
        bias = ((u >> drop) & np.uint32(1)) + np.uint32((1 << (drop - 1)) - 1)
        u = (u + bias) & np.uint32(~((1 << drop) - 1) & 0xFFFFFFFF)
        return u.view(np.float32)

    def lv2(w, scale):
        # 2-level fp8 split: levels (h, m) with the m level scaled by `scale`.
        w = np.asarray(w, f32)
        h = w.astype(fp8)
        m = ((w - h.astype(f32)) * scale).astype(fp8)
        return np.ascontiguousarray(np.stack([h, m], axis=1))

    wq_f = np.asarray(Wq, f32)
    wq8 = np.ascontiguousarray(np.stack(
        [wq_f.astype(fp8),
         ((wq_f - rne11(wq_f)) * 4096.0).astype(fp8)], axis=1))
    wk_f = np.asarray(Wk, f32)
    wv8 = lv2(Wv, 16.0)
    wo_b = np.asarray(Wo, f32).astype(bf16)
    e_sel = np.zeros((2, P), dtype=bf16)
    e_sel[0, :64] = 1
    e_sel[1, 64:] = 1

    in_maps = []
    for c in range(NCORES):
        b, half = c // 2, c % 2
        xs = x[b, half * TOK:(half + 1) * TOK]          # [2048, 1024]
        xtc = np.ascontiguousarray(xs.T)
        m = {
            "xt": xtc,
            "wq": wq_f, "wq8": wq8, "wk": wk_f, "wv8": wv8, "wo": wo_b,
            "x8": lv2(xtc, 1.0),
            "xl8": ((xtc - rne11(xtc)) * 4096.0).astype(fp8),
            "xres": xs.astype(bf16),
            "e_sel": e_sel,
        }
        if apply_bias:
            m.update(bq=np.asarray(bq, f32), bk=np.asarray(bk, f32),
                     bv=np.asarray(bv, f32), bo=np.asarray(bo, f32))
        if apply_gamma:
            m["gamma"] = np.asarray(gamma, f32)
        if apply_beta:
            m["beta"] = np.asarray(beta, f32)
        in_maps.append(m)

    import os
    try:
        LAST_RESULT = run_bass_kernel_spmd(nc, in_maps, core_ids=list(range(NCORES)))
    except ModuleNotFoundError:
        # no antenv.axon_hooks in this container -> NTFF tracing unavailable
        os.environ["BASS_NEVER_TRACE"] = "1"
        LAST_RESULT = run_bass_kernel_spmd(nc, in_maps, core_ids=list(range(NCORES)))
    out = np.empty((B, N, D), dtype=np.float32)
    for c in range(NCORES):
        b, half = c // 2, c % 2
        out[b, half * TOK:(half + 1) * TOK] = LAST_RESULT.results[c]["out"]
    return out
